# revision 19
# baseline (speedup 1.0000x reference)
"""MixerHead kernel for 8 trn2 NeuronCores (Bass/Tile, bf16 matmuls).

Math (reference):
  proj[b,h,l,e]  = sum_d x[b,l,d] Wp[h,e,d] + bp[h,e]
  mixed[b,h,f,e] = sum_{l<=f} Wc[h,f,l] proj[b,h,l,e] + bc[h,f]
  out[b,f,j]     = sum_{h,e} mixed[b,h,f,e] Wo[j, h*E+e] + bo[j]

Sharding: core c = (batch b = c//2, head-pair hp = c%2 -> heads {2hp, 2hp+1}).
Each core computes the bias-free linear part for its (batch, 2 heads) and
writes a partial [L, D] output; host sums the two partials per batch and adds
all bias contributions (folded into a single [L, D] matrix analytically).

Device layout chain (every matmul is out = lhsT.T @ rhs, contraction on the
partition dim):
  phase1: proj[l,e]    lhsT = xT[d, l-tile]          rhs = WpT[d, e(512)]
  phase2: mixedT[e,f]  lhsT = proj[l-tile, e-block]  rhs = WcT[l-tile, f-chunk]
          (WcT is pre-masked tril(Wc).T, packed on host so only lower-tri
           l-tiles are stored/loaded/computed; within a chunk the last 3
           l-tiles use a shrinking free dim to skip the masked-out f-tiles)
  phase3: part[f,dout] lhsT = mixT[e-blk, f-tile]    rhs = WoT[e-blk, dout]

Schedule: phase1 for all 4 l-chunks first, then f-chunks in DESCENDING size
order (3,2,1,0) so the final phase2->phase3->store tail is the smallest
chunk.  All weights (wc both heads, wo) are SBUF-resident, prefetched with
big DMAs issued at kernel start in exactly consumption order.
"""

import sys

for _p in ("/opt/trn_rl_repo", "/root/.axon_site/_ro/trn_rl_repo"):
    if _p not in sys.path:
        sys.path.append(_p)

import numpy as np

import ml_dtypes

try:  # make trace requests degrade gracefully if the NTFF hook module is absent
    import antenv.axon_hooks  # noqa: F401
except ImportError:
    import types

    import antenv

    _m = types.ModuleType("antenv.axon_hooks")
    _h = {}
    _m.set_axon_ntff_profile_hook = lambda hook: _h.__setitem__("h", hook)
    _m.get_axon_ntff_profile_hook = lambda: _h.get("h")
    sys.modules["antenv.axon_hooks"] = _m
    antenv.axon_hooks = _m

from concourse import bacc, mybir, tile
from concourse.bass_utils import run_bass_kernel_spmd

B, L, D, H, E = 4, 2048, 1024, 4, 256
F32 = mybir.dt.float32
BF16 = mybir.dt.bfloat16

LT = L // 128   # 16 l-tiles per batch
FC = 4          # f-chunks of 512
DT8 = D // 128  # 8 d-tiles
CHUNK_ORDER = (3, 2, 1, 0)  # descending causal depth
WC_PACK_COLS = sum((4 * c + 4) * 512 for c in range(FC))  # 20480
# col offset of chunk c's block in the descending-packed wc
WC_OFFS = {}
_off = 0
for _c in CHUNK_ORDER:
    WC_OFFS[_c] = _off
    _off += (4 * _c + 4) * 512

# Set by test harness: run with trace and record exec time.
TRACE = False
LAST_EXEC_NS = None

_cache = {}


def _build_program():
    if "nc" in _cache:
        return _cache["nc"]
    nc = bacc.Bacc("TRN2", target_bir_lowering=False, debug=False, num_devices=8)

    # All inputs pre-tiled on host to the exact SBUF layout ([128, N]) so
    # every DMA is a contiguous column slice (2-8KB per-partition runs ->
    # large descriptors -> full HBM bandwidth during the startup burst).
    xTt = nc.dram_tensor("xTt", [128, 4 * DT8 * 512], BF16, kind="ExternalInput")
    wpTt = nc.dram_tensor("wpTt", [128, DT8 * 512], BF16, kind="ExternalInput")
    wc0 = nc.dram_tensor("wc0", [128, WC_PACK_COLS], BF16, kind="ExternalInput")
    wc1 = nc.dram_tensor("wc1", [128, WC_PACK_COLS], BF16, kind="ExternalInput")
    woTt = nc.dram_tensor("woTt", [128, 4 * D], BF16, kind="ExternalInput")
    part = nc.dram_tensor("part", [L, D], BF16, kind="ExternalOutput")
    wc_dram = [wc0, wc1]

    with tile.TileContext(nc) as tc:
        with (
            tc.tile_pool(name="wp", bufs=1) as wp_pool,
            tc.tile_pool(name="wo", bufs=1) as wo_pool,
            tc.tile_pool(name="xt", bufs=1) as x_pool,
            tc.tile_pool(name="wc", bufs=1) as wc_pool,
            tc.tile_pool(name="proj", bufs=1) as proj_pool,
            tc.tile_pool(name="mix", bufs=1) as mix_pool,
            tc.tile_pool(name="outs", bufs=4) as out_pool,
            tc.tile_pool(name="ps1", bufs=1, space="PSUM") as ps1_pool,
            tc.tile_pool(name="ps2", bufs=2, space="PSUM") as ps2_pool,
            tc.tile_pool(name="ps3", bufs=2, space="PSUM") as ps3_pool,
        ):
            # PE warm-up: dummy matmuls with no DMA dependency run during the
            # startup loads so the HAM clock-gate opens (1.2 -> 2.4 GHz)
            # before the first real matmul.
            warm = wp_pool.tile([128, 512], BF16, tag="warm")
            nc.gpsimd.memset(warm[:], 0.0)
            # Warm-ups bridge from preamble end (~7.6us) past the first
            # DMA-piece arrival (~12us); HAM flips to 2.4GHz ~3.4us in, so
            # the real matmul stream starts fully warm with no early gaps
            # (any gap before the flip restarts the 3.4us busy window).
            ps_w = ps3_pool.tile([128, 512], F32, tag="ps3", name="ps_warm")
            for _ in range(12):
                nc.tensor.matmul(
                    ps_w[:], warm[:, :128], warm[:], start=True, stop=True
                )

            # ---- resident weights / inputs, prefetched in consumption order.
            # The sync HWDGE queue empirically outpaces the scalar one when
            # both are busy, so ALL startup-critical loads (wp + every xt
            # chunk) go on sync, interleaved in exact PE-need order; the
            # scalar queue carries only the late-needed bulk (wc head1, wo).
            wp_all = wp_pool.tile([128, DT8 * 2 * E], BF16, tag="wp")
            xt_tiles = {}

            def load_xt(c, pieces, eng):
                xt_all = x_pool.tile(
                    [128, DT8 * 512], BF16, tag=f"xt{c}", name=f"xt_{c}"
                )
                xt_tiles[c] = xt_all
                for s, w in pieces:
                    eng.dma_start(
                        xt_all[:, s : s + w],
                        xTt[:, c * 4096 + s : c * 4096 + s + w],
                    )

            nc.sync.dma_start(wp_all[:, 0:1024], wpTt[:, 0:1024])  # d0-1
            xt0 = x_pool.tile([128, DT8 * 512], BF16, tag="xt0", name="xt_0")
            xt_tiles[0] = xt0
            nc.sync.dma_start(xt0[:, 0:1024], xTt[:, 0:1024])  # d0-1
            nc.sync.dma_start(wp_all[:, 1024:4096], wpTt[:, 1024:4096])
            nc.sync.dma_start(xt0[:, 1024:4096], xTt[:, 1024:4096])
            wp = [wp_all[:, d * 512 : (d + 1) * 512] for d in range(DT8)]

            load_xt(1, ((0, 2048), (2048, 2048)), nc.sync)
            load_xt(2, ((0, 2048), (2048, 2048)), nc.sync)
            load_xt(3, ((0, 2048), (2048, 2048)), nc.sync)

            wc_sb = [
                wc_pool.tile([128, WC_PACK_COLS], BF16, tag=f"wch{hh}", name=f"wc_{hh}")
                for hh in range(2)
            ]
            # (col_start, col_width) pieces per head: chunk 3 split in two,
            # then chunks 2, 1, 0.
            wc_pieces = [
                (WC_OFFS[3], 4096),
                (WC_OFFS[3] + 4096, 4096),
                (WC_OFFS[2], 6144),
                (WC_OFFS[1], 4096),
                (WC_OFFS[0], 2048),
            ]
            for s, w in wc_pieces:  # head 0 on sync (behind wp + all xt)
                nc.sync.dma_start(wc_sb[0][:, s : s + w], wc_dram[0][:, s : s + w])

            # head 1 chunk-3 pieces, then wo, then the rest (scalar queue)
            for s, w in wc_pieces[:2]:
                nc.scalar.dma_start(wc_sb[1][:, s : s + w], wc_dram[1][:, s : s + w])
            wo_all = wo_pool.tile([128, 4 * D], BF16, tag="wo", name="wo_all")
            nc.scalar.dma_start(wo_all[:], woTt[:, :])
            for s, w in wc_pieces[2:]:
                nc.scalar.dma_start(wc_sb[1][:, s : s + w], wc_dram[1][:, s : s + w])

            proj = [None] * LT
            mix = [[None] * FC for _ in range(4)]

            def phase1(c):
                ps1 = [
                    ps1_pool.tile([128, 2 * E], F32, tag=f"ps1_{i}", name=f"ps1_{c}_{i}")
                    for i in range(4)
                ]
                xt_all = xt_tiles[c]
                for d in range(DT8):
                    for i in range(4):
                        nc.tensor.matmul(
                            ps1[i][:],
                            xt_all[:, d * 512 + i * 128 : d * 512 + (i + 1) * 128],
                            wp[d],
                            start=(d == 0),
                            stop=(d == DT8 - 1),
                        )
                for i in range(4):
                    lt = c * 4 + i
                    pt = proj_pool.tile(
                        [128, 2 * E], BF16, tag=f"proj{lt}", name=f"proj_{lt}"
                    )
                    nc.vector.tensor_copy(pt[:], ps1[i][:])
                    proj[lt] = pt

            def phase2(c):
                # causal => l-tiles 0..4c+3 (mask pre-applied in the packed Wc)
                # l-tile t covers f-tiles >= t-4c within the chunk, so the
                # last three l-tiles shrink their free dim.
                T = 4 * c + 4
                base = WC_OFFS[c]
                for hh in range(2):
                    wct = wc_sb[hh]
                    for eb in (2 * hh, 2 * hh + 1):
                        ps = ps2_pool.tile(
                            [128, 512], F32, tag="ps2", name=f"ps2_{c}_{eb}"
                        )
                        for t in range(T):
                            sh = 128 * max(0, t - 4 * c)
                            nc.tensor.matmul(
                                ps[:, sh:],
                                proj[t][:, eb * 128 : (eb + 1) * 128],
                                wct[:, base + t * 512 + sh : base + (t + 1) * 512],
                                start=(t == 0),
                                stop=(t == T - 1),
                            )
                        mt = mix_pool.tile(
                            [128, 512], BF16, tag=f"m{eb}_{c}", name=f"mix_{eb}_{c}"
                        )
                        # the last chain gates phase3: copy it on the (fast)
                        # vector engine, earlier ones on scalar
                        if eb == 3:
                            nc.vector.tensor_copy(mt[:], ps[:])
                        else:
                            nc.scalar.copy(mt[:], ps[:])
                        mix[eb][c] = mt

            def phase3(c):
                for fi in range(4):
                    ft = c * 4 + fi
                    ot = out_pool.tile([128, D], BF16, tag="out", name=f"out_{ft}")
                    for dc in range(2):
                        ps = ps3_pool.tile(
                            [128, 512], F32, tag="ps3", name=f"ps3_{ft}_{dc}"
                        )
                        for eb in range(4):
                            nc.tensor.matmul(
                                ps[:],
                                mix[eb][c][:, fi * 128 : (fi + 1) * 128],
                                wo_all[
                                    :, eb * D + dc * 512 : eb * D + (dc + 1) * 512
                                ],
                                start=(eb == 0),
                                stop=(eb == 3),
                            )
                        nc.vector.tensor_copy(ot[:, dc * 512 : (dc + 1) * 512], ps[:])
                    nc.scalar.dma_start(part[ft * 128 : (ft + 1) * 128, :], ot[:])

            # Chunk 0 is processed in two 256-f halves so that only ~half a
            # phase3 group remains after the very last phase2 chain -- this
            # shortens the kernel tail (final out-DMA receipt gates the
            # closing barrier).
            mix0 = [[None, None] for _ in range(4)]  # [eb][half]

            def phase2_half0(h):
                base = WC_OFFS[0]
                for hh in range(2):
                    wct = wc_sb[hh]
                    for eb in (2 * hh, 2 * hh + 1):
                        ps = ps2_pool.tile(
                            [128, 256], F32, tag="ps2", name=f"ps2h_{h}_{eb}"
                        )
                        for t in range(2 * h + 2):
                            sh = 128 * max(0, t - 2 * h)
                            nc.tensor.matmul(
                                ps[:, sh:],
                                proj[t][:, eb * 128 : (eb + 1) * 128],
                                wct[
                                    :,
                                    base + t * 512 + 256 * h + sh : base
                                    + t * 512
                                    + 256 * (h + 1),
                                ],
                                start=(t == 0),
                                stop=(t == 2 * h + 1),
                            )
                        mt = mix_pool.tile(
                            [128, 256], BF16, tag=f"m0h{eb}_{h}", name=f"mix0_{eb}_{h}"
                        )
                        if eb == 3:
                            nc.vector.tensor_copy(mt[:], ps[:])
                        else:
                            nc.scalar.copy(mt[:], ps[:])
                        mix0[eb][h] = mt

            def phase3_half0(h):
                for fi in (2 * h, 2 * h + 1):
                    last = h == 1 and fi == 3
                    ot = out_pool.tile([128, D], BF16, tag="out", name=f"out_{fi}")
                    for dc in range(2):
                        if last:
                            # final output: two n=256 chains + split copies and
                            # stores so the very last store issues ASAP after
                            # the last matmul (its HBM receipt gates the
                            # closing barrier)
                            for q in range(2):
                                ps = ps3_pool.tile(
                                    [128, 256], F32, tag="ps3", name=f"ps3f_{dc}_{q}"
                                )
                                for eb in range(4):
                                    nc.tensor.matmul(
                                        ps[:],
                                        mix0[eb][h][:, 128 * (fi - 2 * h) : 128 * (fi - 2 * h + 1)],
                                        wo_all[
                                            :,
                                            eb * D + dc * 512 + q * 256 : eb * D
                                            + dc * 512
                                            + (q + 1) * 256,
                                        ],
                                        start=(eb == 0),
                                        stop=(eb == 3),
                                    )
                                lo = dc * 512 + q * 256
                                nc.vector.tensor_copy(ot[:, lo : lo + 256], ps[:])
                                nc.scalar.dma_start(
                                    part[fi * 128 : (fi + 1) * 128, lo : lo + 256],
                                    ot[:, lo : lo + 256],
                                )
                        else:
                            ps = ps3_pool.tile(
                                [128, 512], F32, tag="ps3", name=f"ps3_{fi}_{dc}"
                            )
                            for eb in range(4):
                                nc.tensor.matmul(
                                    ps[:],
                                    mix0[eb][h][:, (fi - 2 * h) * 128 : (fi - 2 * h + 1) * 128],
                                    wo_all[
                                        :, eb * D + dc * 512 : eb * D + (dc + 1) * 512
                                    ],
                                    start=(eb == 0),
                                    stop=(eb == 3),
                                )
                            nc.vector.tensor_copy(ot[:, dc * 512 : (dc + 1) * 512], ps[:])
                    if not last:
                        nc.scalar.dma_start(part[fi * 128 : (fi + 1) * 128, :], ot[:])

            for c in range(FC):
                phase1(c)
            for c in CHUNK_ORDER[:3]:
                phase2(c)
                phase3(c)
            phase2_half0(0)
            phase3_half0(0)
            phase2_half0(1)
            phase3_half0(1)

    nc.compile()
    _cache["nc"] = nc
    return nc


def _pack_wc_head(wc_h: np.ndarray) -> np.ndarray:
    """tril(Wc[h]) -> [128, 20480]: per f-chunk c (descending order 3,2,1,0),
    the l-tiles 0..4c+3 of WcT = tril(Wc).T laid out as
    [128 l-partitions, T*512 f-cols]."""
    m = np.tril(wc_h)  # [f, l]
    blocks = []
    for c in CHUNK_ORDER:
        T = 4 * c + 4
        sub = m[c * 512 : (c + 1) * 512, : T * 128]  # [512 f, T*128 l]
        subT = sub.T.reshape(T, 128, 512)  # [T, 128 l, 512 f]
        blocks.append(subT.transpose(1, 0, 2).reshape(128, T * 512))
    return np.ascontiguousarray(np.concatenate(blocks, axis=1)).astype(ml_dtypes.bfloat16)


def kernel(x, Wp, bp, Wc, bc, Wo, bo):
    global LAST_EXEC_NS
    x = np.asarray(x, dtype=np.float32)
    Wp = np.asarray(Wp, dtype=np.float32)
    bp = np.asarray(bp, dtype=np.float32)
    Wc = np.asarray(Wc, dtype=np.float32)
    bc = np.asarray(bc, dtype=np.float32)
    Wo = np.asarray(Wo, dtype=np.float32)
    bo = np.asarray(bo, dtype=np.float32)

    nc = _build_program()

    WoT = np.ascontiguousarray(Wo.T)  # [din, dout]
    wc_packed = [_pack_wc_head(Wc[h]) for h in range(H)]
    wpT_pair = []
    woT_pair = []
    for hp in range(2):
        h0, h1 = 2 * hp, 2 * hp + 1
        wpT = np.concatenate([Wp[h0].T, Wp[h1].T], axis=1)  # [D, 2E]
        # pre-tile: wpTt[p, d*512 + e] = wpT[d*128+p, e]
        wpT_pair.append(
            np.ascontiguousarray(
                wpT.reshape(DT8, 128, 2 * E).transpose(1, 0, 2).reshape(128, -1)
            ).astype(ml_dtypes.bfloat16)
        )
        woT_h = np.concatenate(
            [WoT[h0 * E : (h0 + 1) * E], WoT[h1 * E : (h1 + 1) * E]], axis=0
        )  # [2E, D]
        # pre-tile: woTt[p, t*D + j] = woT[t*128+p, j]
        woT_pair.append(
            np.ascontiguousarray(
                woT_h.reshape(4, 128, D).transpose(1, 0, 2).reshape(128, -1)
            ).astype(ml_dtypes.bfloat16)
        )

    # pre-tile x: xTt[p, c*4096 + t*512 + l] = x[b][c*512+l, t*128+p]
    xTt_b = [
        np.ascontiguousarray(
            x[b].T.reshape(DT8, 128, FC, 512).transpose(1, 2, 0, 3).reshape(128, -1)
        ).astype(ml_dtypes.bfloat16)
        for b in range(B)
    ]

    in_maps = []
    for c in range(8):
        b, hp = c // 2, c % 2
        in_maps.append(
            {
                "xTt": xTt_b[b],
                "wpTt": wpT_pair[hp],
                "wc0": wc_packed[2 * hp],
                "wc1": wc_packed[2 * hp + 1],
                "woTt": woT_pair[hp],
            }
        )

    res = run_bass_kernel_spmd(
        nc, in_maps, core_ids=list(range(8)), trace=TRACE
    )
    LAST_EXEC_NS = res.exec_time_ns

    # Host: fold all bias terms into one [L, D] matrix.
    # mixed bias = tril-rowsum(Wc)[h,f] * bp[h,e] + bc[h,f]; through Wo:
    rs = np.tril(Wc).sum(axis=2)  # [H, L]
    Wo_hE = Wo.reshape(D, H, E)
    V = np.einsum("he,jhe->hj", bp, Wo_hE)  # [H, D]
    WoSum = Wo_hE.sum(axis=2)  # [D, H]
    bias_total = rs.T @ V + bc.T @ WoSum.T + bo[None, :]  # [L, D]

    out = np.empty((B, L, D), dtype=np.float32)
    for b in range(B):
        out[b] = (
            res.results[2 * b]["part"].astype(np.float32)
            + res.results[2 * b + 1]["part"].astype(np.float32)
            + bias_total
        )
    return out


# revision 20
# speedup vs baseline: 1.1652x; 1.1652x over previous
"""MixerHead kernel for 8 trn2 NeuronCores (Bass/Tile, bf16 matmuls).

Math (reference):
  proj[b,h,l,e]  = sum_d x[b,l,d] Wp[h,e,d] + bp[h,e]
  mixed[b,h,f,e] = sum_{l<=f} Wc[h,f,l] proj[b,h,l,e] + bc[h,f]
  out[b,f,j]     = sum_{h,e} mixed[b,h,f,e] Wo[j, h*E+e] + bo[j]

Sharding: core c = (batch b = c//2, head-pair hp = c%2 -> heads {2hp, 2hp+1}).
Each core computes the bias-free linear part for its (batch, 2 heads) and
writes a partial [L, D] output; host sums the two partials per batch and adds
all bias contributions (folded into a single [L, D] matrix analytically).

Device layout chain (every matmul is out = lhsT.T @ rhs, contraction on the
partition dim):
  phase1: proj[l,e]    lhsT = xT[d, l-tile]          rhs = WpT[d, e(512)]
  phase2: mixedT[e,f]  lhsT = proj[l-tile, e-block]  rhs = WcT[l-tile, f-chunk]
          (WcT is pre-masked tril(Wc).T, packed on host so only lower-tri
           l-tiles are stored/loaded/computed; within a chunk the last 3
           l-tiles use a shrinking free dim to skip the masked-out f-tiles)
  phase3: part[f,dout] lhsT = mixT[e-blk, f-tile]    rhs = WoT[e-blk, dout]

Schedule: phase1 for all 4 l-chunks first, then f-chunks in DESCENDING size
order (3,2,1,0) so the final phase2->phase3->store tail is the smallest
chunk.  All weights (wc both heads, wo) are SBUF-resident, prefetched with
big DMAs issued at kernel start in exactly consumption order.
"""

import sys

for _p in ("/opt/trn_rl_repo", "/root/.axon_site/_ro/trn_rl_repo"):
    if _p not in sys.path:
        sys.path.append(_p)

import numpy as np

import ml_dtypes

try:  # make trace requests degrade gracefully if the NTFF hook module is absent
    import antenv.axon_hooks  # noqa: F401
except ImportError:
    import types

    import antenv

    _m = types.ModuleType("antenv.axon_hooks")
    _h = {}
    _m.set_axon_ntff_profile_hook = lambda hook: _h.__setitem__("h", hook)
    _m.get_axon_ntff_profile_hook = lambda: _h.get("h")
    sys.modules["antenv.axon_hooks"] = _m
    antenv.axon_hooks = _m

from concourse import bacc, mybir, tile
from concourse.bass_utils import run_bass_kernel_spmd

B, L, D, H, E = 4, 2048, 1024, 4, 256
F32 = mybir.dt.float32
BF16 = mybir.dt.bfloat16

LT = L // 128   # 16 l-tiles per batch
FC = 4          # f-chunks of 512
DT8 = D // 128  # 8 d-tiles
CHUNK_ORDER = (3, 2, 1, 0)  # descending causal depth
WC_PACK_COLS = sum((4 * c + 4) * 512 for c in range(FC))  # 20480
# col offset of chunk c's block in the descending-packed wc
WC_OFFS = {}
_off = 0
for _c in CHUNK_ORDER:
    WC_OFFS[_c] = _off
    _off += (4 * _c + 4) * 512

# Set by test harness: run with trace and record exec time.
TRACE = False
LAST_EXEC_NS = None

_cache = {}


def _build_program():
    if "nc" in _cache:
        return _cache["nc"]
    nc = bacc.Bacc("TRN2", target_bir_lowering=False, debug=False, num_devices=8)

    # All inputs pre-tiled on host to the exact SBUF layout ([128, N]) so
    # every DMA is a contiguous column slice (2-8KB per-partition runs ->
    # large descriptors -> full HBM bandwidth during the startup burst).
    xTt = nc.dram_tensor("xTt", [128, 4 * DT8 * 512], BF16, kind="ExternalInput")
    wpTt = nc.dram_tensor("wpTt", [128, DT8 * 512], BF16, kind="ExternalInput")
    wc0 = nc.dram_tensor("wc0", [128, WC_PACK_COLS], BF16, kind="ExternalInput")
    wc1 = nc.dram_tensor("wc1", [128, WC_PACK_COLS], BF16, kind="ExternalInput")
    woTt = nc.dram_tensor("woTt", [128, 4 * D], BF16, kind="ExternalInput")
    part = nc.dram_tensor("part", [L, D], BF16, kind="ExternalOutput")
    wc_dram = [wc0, wc1]

    with tile.TileContext(nc) as tc:
        with (
            tc.tile_pool(name="wp", bufs=1) as wp_pool,
            tc.tile_pool(name="wo", bufs=1) as wo_pool,
            tc.tile_pool(name="xt", bufs=1) as x_pool,
            tc.tile_pool(name="wc", bufs=1) as wc_pool,
            tc.tile_pool(name="proj", bufs=1) as proj_pool,
            tc.tile_pool(name="mix", bufs=1) as mix_pool,
            tc.tile_pool(name="outs", bufs=4) as out_pool,
            tc.tile_pool(name="ps1", bufs=1, space="PSUM") as ps1_pool,
            tc.tile_pool(name="ps2", bufs=2, space="PSUM") as ps2_pool,
            tc.tile_pool(name="ps3", bufs=2, space="PSUM") as ps3_pool,
        ):
            # PE warm-up: dummy matmuls with no DMA dependency run during the
            # startup loads so the HAM clock-gate opens (1.2 -> 2.4 GHz)
            # before the first real matmul.
            warm = wp_pool.tile([128, 512], BF16, tag="warm")
            nc.gpsimd.memset(warm[:], 0.0)
            # Warm-ups bridge from preamble end (~7.6us) past the first
            # DMA-piece arrival (~12us); HAM flips to 2.4GHz ~3.4us in, so
            # the real matmul stream starts fully warm with no early gaps
            # (any gap before the flip restarts the 3.4us busy window).
            ps_w = ps3_pool.tile([128, 512], F32, tag="ps3", name="ps_warm")
            for _ in range(12):
                nc.tensor.matmul(
                    ps_w[:], warm[:, :128], warm[:], start=True, stop=True
                )

            # ---- resident weights / inputs, prefetched in consumption order.
            # The sync HWDGE queue empirically outpaces the scalar one when
            # both are busy, so ALL startup-critical loads (wp + every xt
            # chunk) go on sync, interleaved in exact PE-need order; the
            # scalar queue carries only the late-needed bulk (wc head1, wo).
            wp_all = wp_pool.tile([128, DT8 * 2 * E], BF16, tag="wp")
            xt_tiles = {}

            def load_xt(c, pieces, eng):
                xt_all = x_pool.tile(
                    [128, DT8 * 512], BF16, tag=f"xt{c}", name=f"xt_{c}"
                )
                xt_tiles[c] = xt_all
                for s, w in pieces:
                    eng.dma_start(
                        xt_all[:, s : s + w],
                        xTt[:, c * 4096 + s : c * 4096 + s + w],
                    )

            nc.sync.dma_start(wp_all[:, 0:1024], wpTt[:, 0:1024])  # d0-1
            xt0 = x_pool.tile([128, DT8 * 512], BF16, tag="xt0", name="xt_0")
            xt_tiles[0] = xt0
            nc.sync.dma_start(xt0[:, 0:1024], xTt[:, 0:1024])  # d0-1
            nc.sync.dma_start(wp_all[:, 1024:4096], wpTt[:, 1024:4096])
            nc.sync.dma_start(xt0[:, 1024:4096], xTt[:, 1024:4096])
            wp = [wp_all[:, d * 512 : (d + 1) * 512] for d in range(DT8)]

            load_xt(1, ((0, 2048), (2048, 2048)), nc.sync)
            load_xt(2, ((0, 2048), (2048, 2048)), nc.sync)
            load_xt(3, ((0, 2048), (2048, 2048)), nc.sync)

            wc_sb = [
                wc_pool.tile([128, WC_PACK_COLS], BF16, tag=f"wch{hh}", name=f"wc_{hh}")
                for hh in range(2)
            ]
            # (col_start, col_width) pieces per head: chunk 3 split in two,
            # then chunks 2, 1, 0.
            wc_pieces = [
                (WC_OFFS[3], 4096),
                (WC_OFFS[3] + 4096, 4096),
                (WC_OFFS[2], 6144),
                (WC_OFFS[1], 4096),
                (WC_OFFS[0], 2048),
            ]
            for s, w in wc_pieces:  # head 0 on sync (behind wp + all xt)
                nc.sync.dma_start(wc_sb[0][:, s : s + w], wc_dram[0][:, s : s + w])

            # Artificial gate: the scalar queue's big-run wc transfers would
            # otherwise crowd out the startup-critical sync-queue loads in
            # the SDMA round-robin (engines drain whole packets; bigger
            # per-partition runs win).  This junk copy makes the first
            # wc_h1 DMA (WAW) wait until xt0 has fully landed, giving the
            # sync queue all 16 engines for the first ~7us.
            nc.vector.tensor_copy(wc_sb[1][:, 0:8], xt_tiles[0][:, 0:8])

            # head 1 chunk-3 pieces, then wo, then the rest (scalar queue)
            for s, w in wc_pieces[:2]:
                nc.scalar.dma_start(wc_sb[1][:, s : s + w], wc_dram[1][:, s : s + w])
            wo_all = wo_pool.tile([128, 4 * D], BF16, tag="wo", name="wo_all")
            nc.scalar.dma_start(wo_all[:], woTt[:, :])
            for s, w in wc_pieces[2:]:
                nc.scalar.dma_start(wc_sb[1][:, s : s + w], wc_dram[1][:, s : s + w])

            proj = [None] * LT
            mix = [[None] * FC for _ in range(4)]

            def phase1(c):
                ps1 = [
                    ps1_pool.tile([128, 2 * E], F32, tag=f"ps1_{i}", name=f"ps1_{c}_{i}")
                    for i in range(4)
                ]
                xt_all = xt_tiles[c]
                for d in range(DT8):
                    for i in range(4):
                        nc.tensor.matmul(
                            ps1[i][:],
                            xt_all[:, d * 512 + i * 128 : d * 512 + (i + 1) * 128],
                            wp[d],
                            start=(d == 0),
                            stop=(d == DT8 - 1),
                        )
                for i in range(4):
                    lt = c * 4 + i
                    pt = proj_pool.tile(
                        [128, 2 * E], BF16, tag=f"proj{lt}", name=f"proj_{lt}"
                    )
                    nc.vector.tensor_copy(pt[:], ps1[i][:])
                    proj[lt] = pt

            def phase2(c):
                # causal => l-tiles 0..4c+3 (mask pre-applied in the packed Wc)
                # l-tile t covers f-tiles >= t-4c within the chunk, so the
                # last three l-tiles shrink their free dim.
                T = 4 * c + 4
                base = WC_OFFS[c]
                for hh in range(2):
                    wct = wc_sb[hh]
                    for eb in (2 * hh, 2 * hh + 1):
                        ps = ps2_pool.tile(
                            [128, 512], F32, tag="ps2", name=f"ps2_{c}_{eb}"
                        )
                        for t in range(T):
                            sh = 128 * max(0, t - 4 * c)
                            nc.tensor.matmul(
                                ps[:, sh:],
                                proj[t][:, eb * 128 : (eb + 1) * 128],
                                wct[:, base + t * 512 + sh : base + (t + 1) * 512],
                                start=(t == 0),
                                stop=(t == T - 1),
                            )
                        mt = mix_pool.tile(
                            [128, 512], BF16, tag=f"m{eb}_{c}", name=f"mix_{eb}_{c}"
                        )
                        # the last chain gates phase3: copy it on the (fast)
                        # vector engine, earlier ones on scalar
                        if eb == 3:
                            nc.vector.tensor_copy(mt[:], ps[:])
                        else:
                            nc.scalar.copy(mt[:], ps[:])
                        mix[eb][c] = mt

            def phase3(c):
                for fi in range(4):
                    ft = c * 4 + fi
                    ot = out_pool.tile([128, D], BF16, tag="out", name=f"out_{ft}")
                    for dc in range(2):
                        ps = ps3_pool.tile(
                            [128, 512], F32, tag="ps3", name=f"ps3_{ft}_{dc}"
                        )
                        for eb in range(4):
                            nc.tensor.matmul(
                                ps[:],
                                mix[eb][c][:, fi * 128 : (fi + 1) * 128],
                                wo_all[
                                    :, eb * D + dc * 512 : eb * D + (dc + 1) * 512
                                ],
                                start=(eb == 0),
                                stop=(eb == 3),
                            )
                        nc.vector.tensor_copy(ot[:, dc * 512 : (dc + 1) * 512], ps[:])
                    nc.scalar.dma_start(part[ft * 128 : (ft + 1) * 128, :], ot[:])

            # Chunk 0 is processed in two 256-f halves so that only ~half a
            # phase3 group remains after the very last phase2 chain -- this
            # shortens the kernel tail (final out-DMA receipt gates the
            # closing barrier).
            mix0 = [[None, None] for _ in range(4)]  # [eb][half]

            def phase2_half0(h):
                base = WC_OFFS[0]
                for hh in range(2):
                    wct = wc_sb[hh]
                    for eb in (2 * hh, 2 * hh + 1):
                        ps = ps2_pool.tile(
                            [128, 256], F32, tag="ps2", name=f"ps2h_{h}_{eb}"
                        )
                        for t in range(2 * h + 2):
                            sh = 128 * max(0, t - 2 * h)
                            nc.tensor.matmul(
                                ps[:, sh:],
                                proj[t][:, eb * 128 : (eb + 1) * 128],
                                wct[
                                    :,
                                    base + t * 512 + 256 * h + sh : base
                                    + t * 512
                                    + 256 * (h + 1),
                                ],
                                start=(t == 0),
                                stop=(t == 2 * h + 1),
                            )
                        mt = mix_pool.tile(
                            [128, 256], BF16, tag=f"m0h{eb}_{h}", name=f"mix0_{eb}_{h}"
                        )
                        if eb == 3:
                            nc.vector.tensor_copy(mt[:], ps[:])
                        else:
                            nc.scalar.copy(mt[:], ps[:])
                        mix0[eb][h] = mt

            def phase3_half0(h):
                for fi in (2 * h, 2 * h + 1):
                    last = h == 1 and fi == 3
                    ot = out_pool.tile([128, D], BF16, tag="out", name=f"out_{fi}")
                    for dc in range(2):
                        if last:
                            # final output: two n=256 chains + split copies and
                            # stores so the very last store issues ASAP after
                            # the last matmul (its HBM receipt gates the
                            # closing barrier)
                            for q in range(2):
                                ps = ps3_pool.tile(
                                    [128, 256], F32, tag="ps3", name=f"ps3f_{dc}_{q}"
                                )
                                for eb in range(4):
                                    nc.tensor.matmul(
                                        ps[:],
                                        mix0[eb][h][:, 128 * (fi - 2 * h) : 128 * (fi - 2 * h + 1)],
                                        wo_all[
                                            :,
                                            eb * D + dc * 512 + q * 256 : eb * D
                                            + dc * 512
                                            + (q + 1) * 256,
                                        ],
                                        start=(eb == 0),
                                        stop=(eb == 3),
                                    )
                                lo = dc * 512 + q * 256
                                nc.vector.tensor_copy(ot[:, lo : lo + 256], ps[:])
                                nc.scalar.dma_start(
                                    part[fi * 128 : (fi + 1) * 128, lo : lo + 256],
                                    ot[:, lo : lo + 256],
                                )
                        else:
                            ps = ps3_pool.tile(
                                [128, 512], F32, tag="ps3", name=f"ps3_{fi}_{dc}"
                            )
                            for eb in range(4):
                                nc.tensor.matmul(
                                    ps[:],
                                    mix0[eb][h][:, (fi - 2 * h) * 128 : (fi - 2 * h + 1) * 128],
                                    wo_all[
                                        :, eb * D + dc * 512 : eb * D + (dc + 1) * 512
                                    ],
                                    start=(eb == 0),
                                    stop=(eb == 3),
                                )
                            nc.vector.tensor_copy(ot[:, dc * 512 : (dc + 1) * 512], ps[:])
                    if not last:
                        nc.scalar.dma_start(part[fi * 128 : (fi + 1) * 128, :], ot[:])

            for c in range(FC):
                phase1(c)
            for c in CHUNK_ORDER[:3]:
                phase2(c)
                phase3(c)
            phase2_half0(0)
            phase3_half0(0)
            phase2_half0(1)
            phase3_half0(1)

    nc.compile()
    _cache["nc"] = nc
    return nc


def _pack_wc_head(wc_h: np.ndarray) -> np.ndarray:
    """tril(Wc[h]) -> [128, 20480]: per f-chunk c (descending order 3,2,1,0),
    the l-tiles 0..4c+3 of WcT = tril(Wc).T laid out as
    [128 l-partitions, T*512 f-cols]."""
    m = np.tril(wc_h)  # [f, l]
    blocks = []
    for c in CHUNK_ORDER:
        T = 4 * c + 4
        sub = m[c * 512 : (c + 1) * 512, : T * 128]  # [512 f, T*128 l]
        subT = sub.T.reshape(T, 128, 512)  # [T, 128 l, 512 f]
        blocks.append(subT.transpose(1, 0, 2).reshape(128, T * 512))
    return np.ascontiguousarray(np.concatenate(blocks, axis=1)).astype(ml_dtypes.bfloat16)


def kernel(x, Wp, bp, Wc, bc, Wo, bo):
    global LAST_EXEC_NS
    x = np.asarray(x, dtype=np.float32)
    Wp = np.asarray(Wp, dtype=np.float32)
    bp = np.asarray(bp, dtype=np.float32)
    Wc = np.asarray(Wc, dtype=np.float32)
    bc = np.asarray(bc, dtype=np.float32)
    Wo = np.asarray(Wo, dtype=np.float32)
    bo = np.asarray(bo, dtype=np.float32)

    nc = _build_program()

    WoT = np.ascontiguousarray(Wo.T)  # [din, dout]
    wc_packed = [_pack_wc_head(Wc[h]) for h in range(H)]
    wpT_pair = []
    woT_pair = []
    for hp in range(2):
        h0, h1 = 2 * hp, 2 * hp + 1
        wpT = np.concatenate([Wp[h0].T, Wp[h1].T], axis=1)  # [D, 2E]
        # pre-tile: wpTt[p, d*512 + e] = wpT[d*128+p, e]
        wpT_pair.append(
            np.ascontiguousarray(
                wpT.reshape(DT8, 128, 2 * E).transpose(1, 0, 2).reshape(128, -1)
            ).astype(ml_dtypes.bfloat16)
        )
        woT_h = np.concatenate(
            [WoT[h0 * E : (h0 + 1) * E], WoT[h1 * E : (h1 + 1) * E]], axis=0
        )  # [2E, D]
        # pre-tile: woTt[p, t*D + j] = woT[t*128+p, j]
        woT_pair.append(
            np.ascontiguousarray(
                woT_h.reshape(4, 128, D).transpose(1, 0, 2).reshape(128, -1)
            ).astype(ml_dtypes.bfloat16)
        )

    # pre-tile x: xTt[p, c*4096 + t*512 + l] = x[b][c*512+l, t*128+p]
    xTt_b = [
        np.ascontiguousarray(
            x[b].T.reshape(DT8, 128, FC, 512).transpose(1, 2, 0, 3).reshape(128, -1)
        ).astype(ml_dtypes.bfloat16)
        for b in range(B)
    ]

    in_maps = []
    for c in range(8):
        b, hp = c // 2, c % 2
        in_maps.append(
            {
                "xTt": xTt_b[b],
                "wpTt": wpT_pair[hp],
                "wc0": wc_packed[2 * hp],
                "wc1": wc_packed[2 * hp + 1],
                "woTt": woT_pair[hp],
            }
        )

    res = run_bass_kernel_spmd(
        nc, in_maps, core_ids=list(range(8)), trace=TRACE
    )
    LAST_EXEC_NS = res.exec_time_ns

    # Host: fold all bias terms into one [L, D] matrix.
    # mixed bias = tril-rowsum(Wc)[h,f] * bp[h,e] + bc[h,f]; through Wo:
    rs = np.tril(Wc).sum(axis=2)  # [H, L]
    Wo_hE = Wo.reshape(D, H, E)
    V = np.einsum("he,jhe->hj", bp, Wo_hE)  # [H, D]
    WoSum = Wo_hE.sum(axis=2)  # [D, H]
    bias_total = rs.T @ V + bc.T @ WoSum.T + bo[None, :]  # [L, D]

    out = np.empty((B, L, D), dtype=np.float32)
    for b in range(B):
        out[b] = (
            res.results[2 * b]["part"].astype(np.float32)
            + res.results[2 * b + 1]["part"].astype(np.float32)
            + bias_total
        )
    return out


# revision 22
# speedup vs baseline: 1.2900x; 1.1071x over previous
"""MixerHead kernel for 8 trn2 NeuronCores (Bass/Tile, bf16 matmuls).

Math (reference):
  proj[b,h,l,e]  = sum_d x[b,l,d] Wp[h,e,d] + bp[h,e]
  mixed[b,h,f,e] = sum_{l<=f} Wc[h,f,l] proj[b,h,l,e] + bc[h,f]
  out[b,f,j]     = sum_{h,e} mixed[b,h,f,e] Wo[j, h*E+e] + bo[j]

Sharding: core c = (batch b = c//2, head-pair hp = c%2 -> heads {2hp, 2hp+1}).
Each core computes the bias-free linear part for its (batch, 2 heads) and
writes a partial [L, D] output; host sums the two partials per batch and adds
all bias contributions (folded into a single [L, D] matrix analytically).

Device layout chain (every matmul is out = lhsT.T @ rhs, contraction on the
partition dim):
  phase1: proj[l,e]    lhsT = xT[d, l-tile]          rhs = WpT[d, e(512)]
  phase2: mixedT[e,f]  lhsT = proj[l-tile, e-block]  rhs = WcT[l-tile, f-chunk]
          (WcT is pre-masked tril(Wc).T, packed on host so only lower-tri
           l-tiles are stored/loaded/computed; within a chunk the last 3
           l-tiles use a shrinking free dim to skip the masked-out f-tiles)
  phase3: part[f,dout] lhsT = mixT[e-blk, f-tile]    rhs = WoT[e-blk, dout]

Schedule: phase1 for all 4 l-chunks first, then f-chunks in DESCENDING size
order (3,2,1,0) so the final phase2->phase3->store tail is the smallest
chunk.  All weights (wc both heads, wo) are SBUF-resident, prefetched with
big DMAs issued at kernel start in exactly consumption order.
"""

import sys

for _p in ("/opt/trn_rl_repo", "/root/.axon_site/_ro/trn_rl_repo"):
    if _p not in sys.path:
        sys.path.append(_p)

import numpy as np

import ml_dtypes

try:  # make trace requests degrade gracefully if the NTFF hook module is absent
    import antenv.axon_hooks  # noqa: F401
except ImportError:
    import types

    import antenv

    _m = types.ModuleType("antenv.axon_hooks")
    _h = {}
    _m.set_axon_ntff_profile_hook = lambda hook: _h.__setitem__("h", hook)
    _m.get_axon_ntff_profile_hook = lambda: _h.get("h")
    sys.modules["antenv.axon_hooks"] = _m
    antenv.axon_hooks = _m

from concourse import bacc, mybir, tile
from concourse.bass_utils import run_bass_kernel_spmd

B, L, D, H, E = 4, 2048, 1024, 4, 256
F32 = mybir.dt.float32
BF16 = mybir.dt.bfloat16

LT = L // 128   # 16 l-tiles per batch
FC = 4          # f-chunks of 512
DT8 = D // 128  # 8 d-tiles
CHUNK_ORDER = (3, 2, 1, 0)  # descending causal depth
WC_PACK_COLS = sum((4 * c + 4) * 512 for c in range(FC))  # 20480
# col offset of chunk c's block in the descending-packed wc
WC_OFFS = {}
_off = 0
for _c in CHUNK_ORDER:
    WC_OFFS[_c] = _off
    _off += (4 * _c + 4) * 512

# Set by test harness: run with trace and record exec time.
TRACE = False
LAST_EXEC_NS = None

_cache = {}


def _build_program():
    if "nc" in _cache:
        return _cache["nc"]
    nc = bacc.Bacc("TRN2", target_bir_lowering=False, debug=False, num_devices=8)

    # All inputs pre-tiled on host to the exact SBUF layout ([128, N]) so
    # every DMA is a contiguous column slice (2-8KB per-partition runs ->
    # large descriptors -> full HBM bandwidth during the startup burst).
    xTt = nc.dram_tensor("xTt", [128, 4 * DT8 * 512], BF16, kind="ExternalInput")
    wpTt = nc.dram_tensor("wpTt", [128, DT8 * 512], BF16, kind="ExternalInput")
    wc0 = nc.dram_tensor("wc0", [128, WC_PACK_COLS], BF16, kind="ExternalInput")
    wc1 = nc.dram_tensor("wc1", [128, WC_PACK_COLS], BF16, kind="ExternalInput")
    woTt = nc.dram_tensor("woTt", [128, 4 * D], BF16, kind="ExternalInput")
    part = nc.dram_tensor("part", [L, D], BF16, kind="ExternalOutput")
    wc_dram = [wc0, wc1]

    with tile.TileContext(nc) as tc:
        with (
            tc.tile_pool(name="wp", bufs=1) as wp_pool,
            tc.tile_pool(name="wo", bufs=1) as wo_pool,
            tc.tile_pool(name="xt", bufs=1) as x_pool,
            tc.tile_pool(name="wc", bufs=1) as wc_pool,
            tc.tile_pool(name="proj", bufs=1) as proj_pool,
            tc.tile_pool(name="mix", bufs=1) as mix_pool,
            tc.tile_pool(name="outs", bufs=4) as out_pool,
            tc.tile_pool(name="ps1", bufs=1, space="PSUM") as ps1_pool,
            tc.tile_pool(name="ps2", bufs=2, space="PSUM") as ps2_pool,
            tc.tile_pool(name="ps3", bufs=2, space="PSUM") as ps3_pool,
        ):
            # PE warm-up: dummy matmuls with no DMA dependency run during the
            # startup loads so the HAM clock-gate opens (1.2 -> 2.4 GHz)
            # before the first real matmul.
            warm = wp_pool.tile([128, 512], BF16, tag="warm")
            nc.gpsimd.memset(warm[:], 0.0)
            # Warm-ups bridge from preamble end (~7.6us) past the first
            # DMA-piece arrival (~12us); HAM flips to 2.4GHz ~3.4us in, so
            # the real matmul stream starts fully warm with no early gaps
            # (any gap before the flip restarts the 3.4us busy window).
            ps_w = ps3_pool.tile([128, 512], F32, tag="ps3", name="ps_warm")
            for _ in range(12):
                nc.tensor.matmul(
                    ps_w[:], warm[:, :128], warm[:], start=True, stop=True
                )

            # ---- resident weights / inputs, prefetched in consumption order.
            # Two HWDGE queues (sync, scalar); pieces assigned so each queue
            # delivers in PE-need order.
            wp_all = wp_pool.tile([128, DT8 * 2 * E], BF16, tag="wp")
            for s, w in ((0, 2048), (2048, 2048)):  # d0-3, d4-7
                nc.sync.dma_start(wp_all[:, s : s + w], wpTt[:, s : s + w])
            wp = [wp_all[:, d * 512 : (d + 1) * 512] for d in range(DT8)]

            xt_tiles = {}

            def load_xt(c, pieces, eng):
                xt_all = x_pool.tile(
                    [128, DT8 * 512], BF16, tag=f"xt{c}", name=f"xt_{c}"
                )
                xt_tiles[c] = xt_all
                for s, w in pieces:
                    eng.dma_start(
                        xt_all[:, s : s + w],
                        xTt[:, c * 4096 + s : c * 4096 + s + w],
                    )

            load_xt(0, ((0, 2048), (2048, 2048)), nc.scalar)
            load_xt(1, ((0, 2048), (2048, 2048)), nc.sync)
            load_xt(2, ((0, 2048), (2048, 2048)), nc.scalar)
            load_xt(3, ((0, 2048), (2048, 2048)), nc.scalar)

            wc_sb = [
                wc_pool.tile([128, WC_PACK_COLS], BF16, tag=f"wch{hh}", name=f"wc_{hh}")
                for hh in range(2)
            ]
            # (col_start, col_width) pieces per head: chunk 3 split in two,
            # then chunks 2, 1, 0.
            wc_pieces = [
                (WC_OFFS[3], 4096),
                (WC_OFFS[3] + 4096, 4096),
                (WC_OFFS[2], 6144),
                (WC_OFFS[1], 4096),
                (WC_OFFS[0], 2048),
            ]
            for s, w in wc_pieces:  # head 0 on sync (behind wp + all xt)
                nc.sync.dma_start(wc_sb[0][:, s : s + w], wc_dram[0][:, s : s + w])

            # head 1 chunk-3 pieces, then wo, then the rest (scalar queue)
            for s, w in wc_pieces[:2]:
                nc.scalar.dma_start(wc_sb[1][:, s : s + w], wc_dram[1][:, s : s + w])
            wo_all = wo_pool.tile([128, 4 * D], BF16, tag="wo", name="wo_all")
            nc.scalar.dma_start(wo_all[:], woTt[:, :])
            for s, w in wc_pieces[2:]:
                nc.scalar.dma_start(wc_sb[1][:, s : s + w], wc_dram[1][:, s : s + w])

            proj = [None] * LT
            mix = [[None] * FC for _ in range(4)]

            def phase1(c):
                ps1 = [
                    ps1_pool.tile([128, 2 * E], F32, tag=f"ps1_{i}", name=f"ps1_{c}_{i}")
                    for i in range(4)
                ]
                xt_all = xt_tiles[c]
                for d in range(DT8):
                    for i in range(4):
                        nc.tensor.matmul(
                            ps1[i][:],
                            xt_all[:, d * 512 + i * 128 : d * 512 + (i + 1) * 128],
                            wp[d],
                            start=(d == 0),
                            stop=(d == DT8 - 1),
                        )
                for i in range(4):
                    lt = c * 4 + i
                    pt = proj_pool.tile(
                        [128, 2 * E], BF16, tag=f"proj{lt}", name=f"proj_{lt}"
                    )
                    nc.vector.tensor_copy(pt[:], ps1[i][:])
                    proj[lt] = pt

            def phase2(c):
                # causal => l-tiles 0..4c+3 (mask pre-applied in the packed Wc)
                # l-tile t covers f-tiles >= t-4c within the chunk, so the
                # last three l-tiles shrink their free dim.
                T = 4 * c + 4
                base = WC_OFFS[c]
                for hh in range(2):
                    wct = wc_sb[hh]
                    for eb in (2 * hh, 2 * hh + 1):
                        ps = ps2_pool.tile(
                            [128, 512], F32, tag="ps2", name=f"ps2_{c}_{eb}"
                        )
                        for t in range(T):
                            sh = 128 * max(0, t - 4 * c)
                            nc.tensor.matmul(
                                ps[:, sh:],
                                proj[t][:, eb * 128 : (eb + 1) * 128],
                                wct[:, base + t * 512 + sh : base + (t + 1) * 512],
                                start=(t == 0),
                                stop=(t == T - 1),
                            )
                        mt = mix_pool.tile(
                            [128, 512], BF16, tag=f"m{eb}_{c}", name=f"mix_{eb}_{c}"
                        )
                        # the last chain gates phase3: copy it on the (fast)
                        # vector engine, earlier ones on scalar
                        if eb == 3:
                            nc.vector.tensor_copy(mt[:], ps[:])
                        else:
                            nc.scalar.copy(mt[:], ps[:])
                        mix[eb][c] = mt

            def phase3(c):
                for fi in range(4):
                    ft = c * 4 + fi
                    ot = out_pool.tile([128, D], BF16, tag="out", name=f"out_{ft}")
                    for dc in range(2):
                        ps = ps3_pool.tile(
                            [128, 512], F32, tag="ps3", name=f"ps3_{ft}_{dc}"
                        )
                        for eb in range(4):
                            nc.tensor.matmul(
                                ps[:],
                                mix[eb][c][:, fi * 128 : (fi + 1) * 128],
                                wo_all[
                                    :, eb * D + dc * 512 : eb * D + (dc + 1) * 512
                                ],
                                start=(eb == 0),
                                stop=(eb == 3),
                            )
                        nc.vector.tensor_copy(ot[:, dc * 512 : (dc + 1) * 512], ps[:])
                    nc.scalar.dma_start(part[ft * 128 : (ft + 1) * 128, :], ot[:])

            # Chunk 0 is processed in two 256-f halves so that only ~half a
            # phase3 group remains after the very last phase2 chain -- this
            # shortens the kernel tail (final out-DMA receipt gates the
            # closing barrier).
            mix0 = [[None, None] for _ in range(4)]  # [eb][half]

            def phase2_half0(h):
                base = WC_OFFS[0]
                for hh in range(2):
                    wct = wc_sb[hh]
                    for eb in (2 * hh, 2 * hh + 1):
                        ps = ps2_pool.tile(
                            [128, 256], F32, tag="ps2", name=f"ps2h_{h}_{eb}"
                        )
                        for t in range(2 * h + 2):
                            sh = 128 * max(0, t - 2 * h)
                            nc.tensor.matmul(
                                ps[:, sh:],
                                proj[t][:, eb * 128 : (eb + 1) * 128],
                                wct[
                                    :,
                                    base + t * 512 + 256 * h + sh : base
                                    + t * 512
                                    + 256 * (h + 1),
                                ],
                                start=(t == 0),
                                stop=(t == 2 * h + 1),
                            )
                        mt = mix_pool.tile(
                            [128, 256], BF16, tag=f"m0h{eb}_{h}", name=f"mix0_{eb}_{h}"
                        )
                        if eb == 3:
                            nc.vector.tensor_copy(mt[:], ps[:])
                        else:
                            nc.scalar.copy(mt[:], ps[:])
                        mix0[eb][h] = mt

            def phase3_half0(h):
                for fi in (2 * h, 2 * h + 1):
                    last = h == 1 and fi == 3
                    ot = out_pool.tile([128, D], BF16, tag="out", name=f"out_{fi}")
                    for dc in range(2):
                        if last:
                            # final output: two n=256 chains + split copies and
                            # stores so the very last store issues ASAP after
                            # the last matmul (its HBM receipt gates the
                            # closing barrier)
                            for q in range(2):
                                ps = ps3_pool.tile(
                                    [128, 256], F32, tag="ps3", name=f"ps3f_{dc}_{q}"
                                )
                                for eb in range(4):
                                    nc.tensor.matmul(
                                        ps[:],
                                        mix0[eb][h][:, 128 * (fi - 2 * h) : 128 * (fi - 2 * h + 1)],
                                        wo_all[
                                            :,
                                            eb * D + dc * 512 + q * 256 : eb * D
                                            + dc * 512
                                            + (q + 1) * 256,
                                        ],
                                        start=(eb == 0),
                                        stop=(eb == 3),
                                    )
                                lo = dc * 512 + q * 256
                                nc.vector.tensor_copy(ot[:, lo : lo + 256], ps[:])
                                nc.scalar.dma_start(
                                    part[fi * 128 : (fi + 1) * 128, lo : lo + 256],
                                    ot[:, lo : lo + 256],
                                )
                        else:
                            ps = ps3_pool.tile(
                                [128, 512], F32, tag="ps3", name=f"ps3_{fi}_{dc}"
                            )
                            for eb in range(4):
                                nc.tensor.matmul(
                                    ps[:],
                                    mix0[eb][h][:, (fi - 2 * h) * 128 : (fi - 2 * h + 1) * 128],
                                    wo_all[
                                        :, eb * D + dc * 512 : eb * D + (dc + 1) * 512
                                    ],
                                    start=(eb == 0),
                                    stop=(eb == 3),
                                )
                            nc.vector.tensor_copy(ot[:, dc * 512 : (dc + 1) * 512], ps[:])
                    if not last:
                        nc.scalar.dma_start(part[fi * 128 : (fi + 1) * 128, :], ot[:])

            for c in range(FC):
                phase1(c)
            for c in CHUNK_ORDER[:3]:
                phase2(c)
                phase3(c)
            phase2_half0(0)
            phase3_half0(0)
            phase2_half0(1)
            phase3_half0(1)

    nc.compile()
    _cache["nc"] = nc
    return nc


def _pack_wc_head(wc_h: np.ndarray) -> np.ndarray:
    """tril(Wc[h]) -> [128, 20480]: per f-chunk c (descending order 3,2,1,0),
    the l-tiles 0..4c+3 of WcT = tril(Wc).T laid out as
    [128 l-partitions, T*512 f-cols]."""
    m = np.tril(wc_h)  # [f, l]
    blocks = []
    for c in CHUNK_ORDER:
        T = 4 * c + 4
        sub = m[c * 512 : (c + 1) * 512, : T * 128]  # [512 f, T*128 l]
        subT = sub.T.reshape(T, 128, 512)  # [T, 128 l, 512 f]
        blocks.append(subT.transpose(1, 0, 2).reshape(128, T * 512))
    return np.ascontiguousarray(np.concatenate(blocks, axis=1)).astype(ml_dtypes.bfloat16)


def kernel(x, Wp, bp, Wc, bc, Wo, bo):
    global LAST_EXEC_NS
    x = np.asarray(x, dtype=np.float32)
    Wp = np.asarray(Wp, dtype=np.float32)
    bp = np.asarray(bp, dtype=np.float32)
    Wc = np.asarray(Wc, dtype=np.float32)
    bc = np.asarray(bc, dtype=np.float32)
    Wo = np.asarray(Wo, dtype=np.float32)
    bo = np.asarray(bo, dtype=np.float32)

    nc = _build_program()

    WoT = np.ascontiguousarray(Wo.T)  # [din, dout]
    wc_packed = [_pack_wc_head(Wc[h]) for h in range(H)]
    wpT_pair = []
    woT_pair = []
    for hp in range(2):
        h0, h1 = 2 * hp, 2 * hp + 1
        wpT = np.concatenate([Wp[h0].T, Wp[h1].T], axis=1)  # [D, 2E]
        # pre-tile: wpTt[p, d*512 + e] = wpT[d*128+p, e]
        wpT_pair.append(
            np.ascontiguousarray(
                wpT.reshape(DT8, 128, 2 * E).transpose(1, 0, 2).reshape(128, -1)
            ).astype(ml_dtypes.bfloat16)
        )
        woT_h = np.concatenate(
            [WoT[h0 * E : (h0 + 1) * E], WoT[h1 * E : (h1 + 1) * E]], axis=0
        )  # [2E, D]
        # pre-tile: woTt[p, t*D + j] = woT[t*128+p, j]
        woT_pair.append(
            np.ascontiguousarray(
                woT_h.reshape(4, 128, D).transpose(1, 0, 2).reshape(128, -1)
            ).astype(ml_dtypes.bfloat16)
        )

    # pre-tile x: xTt[p, c*4096 + t*512 + l] = x[b][c*512+l, t*128+p]
    xTt_b = [
        np.ascontiguousarray(
            x[b].T.reshape(DT8, 128, FC, 512).transpose(1, 2, 0, 3).reshape(128, -1)
        ).astype(ml_dtypes.bfloat16)
        for b in range(B)
    ]

    in_maps = []
    for c in range(8):
        b, hp = c // 2, c % 2
        in_maps.append(
            {
                "xTt": xTt_b[b],
                "wpTt": wpT_pair[hp],
                "wc0": wc_packed[2 * hp],
                "wc1": wc_packed[2 * hp + 1],
                "woTt": woT_pair[hp],
            }
        )

    res = run_bass_kernel_spmd(
        nc, in_maps, core_ids=list(range(8)), trace=TRACE
    )
    LAST_EXEC_NS = res.exec_time_ns

    # Host: fold all bias terms into one [L, D] matrix.
    # mixed bias = tril-rowsum(Wc)[h,f] * bp[h,e] + bc[h,f]; through Wo:
    rs = np.tril(Wc).sum(axis=2)  # [H, L]
    Wo_hE = Wo.reshape(D, H, E)
    V = np.einsum("he,jhe->hj", bp, Wo_hE)  # [H, D]
    WoSum = Wo_hE.sum(axis=2)  # [D, H]
    bias_total = rs.T @ V + bc.T @ WoSum.T + bo[None, :]  # [L, D]

    out = np.empty((B, L, D), dtype=np.float32)
    for b in range(B):
        out[b] = (
            res.results[2 * b]["part"].astype(np.float32)
            + res.results[2 * b + 1]["part"].astype(np.float32)
            + bias_total
        )
    return out


# revision 23
# speedup vs baseline: 1.2938x; 1.0029x over previous
"""MixerHead kernel for 8 trn2 NeuronCores (Bass/Tile, bf16 matmuls).

Math (reference):
  proj[b,h,l,e]  = sum_d x[b,l,d] Wp[h,e,d] + bp[h,e]
  mixed[b,h,f,e] = sum_{l<=f} Wc[h,f,l] proj[b,h,l,e] + bc[h,f]
  out[b,f,j]     = sum_{h,e} mixed[b,h,f,e] Wo[j, h*E+e] + bo[j]

Sharding: core c = (batch b = c//2, head-pair hp = c%2 -> heads {2hp, 2hp+1}).
Each core computes the bias-free linear part for its (batch, 2 heads) and
writes a partial [L, D] output; host sums the two partials per batch and adds
all bias contributions (folded into a single [L, D] matrix analytically).

Device layout chain (every matmul is out = lhsT.T @ rhs, contraction on the
partition dim):
  phase1: proj[l,e]    lhsT = xT[d, l-tile]          rhs = WpT[d, e(512)]
  phase2: mixedT[e,f]  lhsT = proj[l-tile, e-block]  rhs = WcT[l-tile, f-chunk]
          (WcT is pre-masked tril(Wc).T, packed on host so only lower-tri
           l-tiles are stored/loaded/computed; within a chunk the last 3
           l-tiles use a shrinking free dim to skip the masked-out f-tiles)
  phase3: part[f,dout] lhsT = mixT[e-blk, f-tile]    rhs = WoT[e-blk, dout]

Schedule: phase1 for all 4 l-chunks first, then f-chunks in DESCENDING size
order (3,2,1,0) so the final phase2->phase3->store tail is the smallest
chunk.  All weights (wc both heads, wo) are SBUF-resident, prefetched with
big DMAs issued at kernel start in exactly consumption order.
"""

import sys

for _p in ("/opt/trn_rl_repo", "/root/.axon_site/_ro/trn_rl_repo"):
    if _p not in sys.path:
        sys.path.append(_p)

import numpy as np

import ml_dtypes

try:  # make trace requests degrade gracefully if the NTFF hook module is absent
    import antenv.axon_hooks  # noqa: F401
except ImportError:
    import types

    import antenv

    _m = types.ModuleType("antenv.axon_hooks")
    _h = {}
    _m.set_axon_ntff_profile_hook = lambda hook: _h.__setitem__("h", hook)
    _m.get_axon_ntff_profile_hook = lambda: _h.get("h")
    sys.modules["antenv.axon_hooks"] = _m
    antenv.axon_hooks = _m

from concourse import bacc, mybir, tile
from concourse.bass_utils import run_bass_kernel_spmd

B, L, D, H, E = 4, 2048, 1024, 4, 256
F32 = mybir.dt.float32
BF16 = mybir.dt.bfloat16

LT = L // 128   # 16 l-tiles per batch
FC = 4          # f-chunks of 512
DT8 = D // 128  # 8 d-tiles
CHUNK_ORDER = (3, 2, 1, 0)  # descending causal depth
WC_PACK_COLS = sum((4 * c + 4) * 512 for c in range(FC))  # 20480
# col offset of chunk c's block in the descending-packed wc
WC_OFFS = {}
_off = 0
for _c in CHUNK_ORDER:
    WC_OFFS[_c] = _off
    _off += (4 * _c + 4) * 512

# Set by test harness: run with trace and record exec time.
TRACE = False
LAST_EXEC_NS = None

_cache = {}


def _build_program():
    if "nc" in _cache:
        return _cache["nc"]
    nc = bacc.Bacc("TRN2", target_bir_lowering=False, debug=False, num_devices=8)

    # All inputs pre-tiled on host to the exact SBUF layout ([128, N]) so
    # every DMA is a contiguous column slice (2-8KB per-partition runs ->
    # large descriptors -> full HBM bandwidth during the startup burst).
    xTt = nc.dram_tensor("xTt", [128, 4 * DT8 * 512], BF16, kind="ExternalInput")
    wpTt = nc.dram_tensor("wpTt", [128, DT8 * 512], BF16, kind="ExternalInput")
    wc0 = nc.dram_tensor("wc0", [128, WC_PACK_COLS], BF16, kind="ExternalInput")
    wc1 = nc.dram_tensor("wc1", [128, WC_PACK_COLS], BF16, kind="ExternalInput")
    woTt = nc.dram_tensor("woTt", [128, 4 * D], BF16, kind="ExternalInput")
    part = nc.dram_tensor("part", [L, D], BF16, kind="ExternalOutput")
    wc_dram = [wc0, wc1]

    with tile.TileContext(nc) as tc:
        with (
            tc.tile_pool(name="wp", bufs=1) as wp_pool,
            tc.tile_pool(name="wo", bufs=1) as wo_pool,
            tc.tile_pool(name="xt", bufs=1) as x_pool,
            tc.tile_pool(name="wc", bufs=1) as wc_pool,
            tc.tile_pool(name="proj", bufs=1) as proj_pool,
            tc.tile_pool(name="mix", bufs=1) as mix_pool,
            tc.tile_pool(name="outs", bufs=4) as out_pool,
            tc.tile_pool(name="ps1", bufs=1, space="PSUM") as ps1_pool,
            tc.tile_pool(name="ps2", bufs=2, space="PSUM") as ps2_pool,
            tc.tile_pool(name="ps3", bufs=2, space="PSUM") as ps3_pool,
        ):
            # PE warm-up: dummy matmuls with no DMA dependency run during the
            # startup loads so the HAM clock-gate opens (1.2 -> 2.4 GHz)
            # before the first real matmul.
            warm = wp_pool.tile([128, 512], BF16, tag="warm")
            nc.gpsimd.memset(warm[:], 0.0)
            # Warm-ups bridge from preamble end (~7.6us) past the first
            # DMA-piece arrival (~12us); HAM flips to 2.4GHz ~3.4us in, so
            # the real matmul stream starts fully warm with no early gaps
            # (any gap before the flip restarts the 3.4us busy window).
            ps_w = ps3_pool.tile([128, 512], F32, tag="ps3", name="ps_warm")
            for _ in range(12):
                nc.tensor.matmul(
                    ps_w[:], warm[:, :128], warm[:], start=True, stop=True
                )

            # ---- resident weights / inputs, prefetched in consumption order.
            # Two HWDGE queues (sync, scalar); pieces assigned so each queue
            # delivers in PE-need order.
            wp_all = wp_pool.tile([128, DT8 * 2 * E], BF16, tag="wp")
            for s, w in ((0, 1024), (1024, 1024), (2048, 2048)):  # d0-1, d2-3, d4-7
                nc.sync.dma_start(wp_all[:, s : s + w], wpTt[:, s : s + w])
            wp = [wp_all[:, d * 512 : (d + 1) * 512] for d in range(DT8)]

            xt_tiles = {}

            def load_xt(c, pieces, eng):
                xt_all = x_pool.tile(
                    [128, DT8 * 512], BF16, tag=f"xt{c}", name=f"xt_{c}"
                )
                xt_tiles[c] = xt_all
                for s, w in pieces:
                    eng.dma_start(
                        xt_all[:, s : s + w],
                        xTt[:, c * 4096 + s : c * 4096 + s + w],
                    )

            load_xt(0, ((0, 1024), (1024, 1024), (2048, 2048)), nc.scalar)
            load_xt(1, ((0, 2048), (2048, 2048)), nc.sync)
            load_xt(2, ((0, 2048), (2048, 2048)), nc.scalar)
            load_xt(3, ((0, 2048), (2048, 2048)), nc.scalar)

            wc_sb = [
                wc_pool.tile([128, WC_PACK_COLS], BF16, tag=f"wch{hh}", name=f"wc_{hh}")
                for hh in range(2)
            ]
            # (col_start, col_width) pieces per head: chunk 3 split in two,
            # then chunks 2, 1, 0.
            wc_pieces = [
                (WC_OFFS[3], 4096),
                (WC_OFFS[3] + 4096, 4096),
                (WC_OFFS[2], 6144),
                (WC_OFFS[1], 4096),
                (WC_OFFS[0], 2048),
            ]
            for s, w in wc_pieces:  # head 0 on sync (behind wp + all xt)
                nc.sync.dma_start(wc_sb[0][:, s : s + w], wc_dram[0][:, s : s + w])

            # head 1 chunk-3 pieces, then wo, then the rest (scalar queue)
            for s, w in wc_pieces[:2]:
                nc.scalar.dma_start(wc_sb[1][:, s : s + w], wc_dram[1][:, s : s + w])
            wo_all = wo_pool.tile([128, 4 * D], BF16, tag="wo", name="wo_all")
            nc.scalar.dma_start(wo_all[:], woTt[:, :])
            for s, w in wc_pieces[2:]:
                nc.scalar.dma_start(wc_sb[1][:, s : s + w], wc_dram[1][:, s : s + w])

            proj = [None] * LT
            mix = [[None] * FC for _ in range(4)]

            def phase1(c):
                ps1 = [
                    ps1_pool.tile([128, 2 * E], F32, tag=f"ps1_{i}", name=f"ps1_{c}_{i}")
                    for i in range(4)
                ]
                xt_all = xt_tiles[c]
                for d in range(DT8):
                    for i in range(4):
                        nc.tensor.matmul(
                            ps1[i][:],
                            xt_all[:, d * 512 + i * 128 : d * 512 + (i + 1) * 128],
                            wp[d],
                            start=(d == 0),
                            stop=(d == DT8 - 1),
                        )
                for i in range(4):
                    lt = c * 4 + i
                    pt = proj_pool.tile(
                        [128, 2 * E], BF16, tag=f"proj{lt}", name=f"proj_{lt}"
                    )
                    nc.vector.tensor_copy(pt[:], ps1[i][:])
                    proj[lt] = pt

            def phase2(c):
                # causal => l-tiles 0..4c+3 (mask pre-applied in the packed Wc)
                # l-tile t covers f-tiles >= t-4c within the chunk, so the
                # last three l-tiles shrink their free dim.
                T = 4 * c + 4
                base = WC_OFFS[c]
                for hh in range(2):
                    wct = wc_sb[hh]
                    for eb in (2 * hh, 2 * hh + 1):
                        ps = ps2_pool.tile(
                            [128, 512], F32, tag="ps2", name=f"ps2_{c}_{eb}"
                        )
                        for t in range(T):
                            sh = 128 * max(0, t - 4 * c)
                            nc.tensor.matmul(
                                ps[:, sh:],
                                proj[t][:, eb * 128 : (eb + 1) * 128],
                                wct[:, base + t * 512 + sh : base + (t + 1) * 512],
                                start=(t == 0),
                                stop=(t == T - 1),
                            )
                        mt = mix_pool.tile(
                            [128, 512], BF16, tag=f"m{eb}_{c}", name=f"mix_{eb}_{c}"
                        )
                        # the last chain gates phase3: copy it on the (fast)
                        # vector engine, earlier ones on scalar
                        if eb == 3:
                            nc.vector.tensor_copy(mt[:], ps[:])
                        else:
                            nc.scalar.copy(mt[:], ps[:])
                        mix[eb][c] = mt

            def phase3(c):
                for fi in range(4):
                    ft = c * 4 + fi
                    ot = out_pool.tile([128, D], BF16, tag="out", name=f"out_{ft}")
                    for dc in range(2):
                        ps = ps3_pool.tile(
                            [128, 512], F32, tag="ps3", name=f"ps3_{ft}_{dc}"
                        )
                        for eb in range(4):
                            nc.tensor.matmul(
                                ps[:],
                                mix[eb][c][:, fi * 128 : (fi + 1) * 128],
                                wo_all[
                                    :, eb * D + dc * 512 : eb * D + (dc + 1) * 512
                                ],
                                start=(eb == 0),
                                stop=(eb == 3),
                            )
                        nc.vector.tensor_copy(ot[:, dc * 512 : (dc + 1) * 512], ps[:])
                    nc.scalar.dma_start(part[ft * 128 : (ft + 1) * 128, :], ot[:])

            # Chunk 0 is processed in two 256-f halves so that only ~half a
            # phase3 group remains after the very last phase2 chain -- this
            # shortens the kernel tail (final out-DMA receipt gates the
            # closing barrier).
            mix0 = [[None, None] for _ in range(4)]  # [eb][half]

            def phase2_half0(h):
                base = WC_OFFS[0]
                for hh in range(2):
                    wct = wc_sb[hh]
                    for eb in (2 * hh, 2 * hh + 1):
                        ps = ps2_pool.tile(
                            [128, 256], F32, tag="ps2", name=f"ps2h_{h}_{eb}"
                        )
                        for t in range(2 * h + 2):
                            sh = 128 * max(0, t - 2 * h)
                            nc.tensor.matmul(
                                ps[:, sh:],
                                proj[t][:, eb * 128 : (eb + 1) * 128],
                                wct[
                                    :,
                                    base + t * 512 + 256 * h + sh : base
                                    + t * 512
                                    + 256 * (h + 1),
                                ],
                                start=(t == 0),
                                stop=(t == 2 * h + 1),
                            )
                        mt = mix_pool.tile(
                            [128, 256], BF16, tag=f"m0h{eb}_{h}", name=f"mix0_{eb}_{h}"
                        )
                        if eb == 3:
                            nc.vector.tensor_copy(mt[:], ps[:])
                        else:
                            nc.scalar.copy(mt[:], ps[:])
                        mix0[eb][h] = mt

            def phase3_half0(h):
                for fi in (2 * h, 2 * h + 1):
                    last = h == 1 and fi == 3
                    ot = out_pool.tile([128, D], BF16, tag="out", name=f"out_{fi}")
                    for dc in range(2):
                        if last:
                            # final output: two n=256 chains + split copies and
                            # stores so the very last store issues ASAP after
                            # the last matmul (its HBM receipt gates the
                            # closing barrier)
                            for q in range(2):
                                ps = ps3_pool.tile(
                                    [128, 256], F32, tag="ps3", name=f"ps3f_{dc}_{q}"
                                )
                                for eb in range(4):
                                    nc.tensor.matmul(
                                        ps[:],
                                        mix0[eb][h][:, 128 * (fi - 2 * h) : 128 * (fi - 2 * h + 1)],
                                        wo_all[
                                            :,
                                            eb * D + dc * 512 + q * 256 : eb * D
                                            + dc * 512
                                            + (q + 1) * 256,
                                        ],
                                        start=(eb == 0),
                                        stop=(eb == 3),
                                    )
                                lo = dc * 512 + q * 256
                                nc.vector.tensor_copy(ot[:, lo : lo + 256], ps[:])
                                nc.scalar.dma_start(
                                    part[fi * 128 : (fi + 1) * 128, lo : lo + 256],
                                    ot[:, lo : lo + 256],
                                )
                        else:
                            ps = ps3_pool.tile(
                                [128, 512], F32, tag="ps3", name=f"ps3_{fi}_{dc}"
                            )
                            for eb in range(4):
                                nc.tensor.matmul(
                                    ps[:],
                                    mix0[eb][h][:, (fi - 2 * h) * 128 : (fi - 2 * h + 1) * 128],
                                    wo_all[
                                        :, eb * D + dc * 512 : eb * D + (dc + 1) * 512
                                    ],
                                    start=(eb == 0),
                                    stop=(eb == 3),
                                )
                            nc.vector.tensor_copy(ot[:, dc * 512 : (dc + 1) * 512], ps[:])
                    if not last:
                        nc.scalar.dma_start(part[fi * 128 : (fi + 1) * 128, :], ot[:])

            for c in range(FC):
                phase1(c)
            for c in CHUNK_ORDER[:3]:
                phase2(c)
                phase3(c)
            phase2_half0(0)
            phase3_half0(0)
            phase2_half0(1)
            phase3_half0(1)

    nc.compile()
    _cache["nc"] = nc
    return nc


def _pack_wc_head(wc_h: np.ndarray) -> np.ndarray:
    """tril(Wc[h]) -> [128, 20480]: per f-chunk c (descending order 3,2,1,0),
    the l-tiles 0..4c+3 of WcT = tril(Wc).T laid out as
    [128 l-partitions, T*512 f-cols]."""
    m = np.tril(wc_h)  # [f, l]
    blocks = []
    for c in CHUNK_ORDER:
        T = 4 * c + 4
        sub = m[c * 512 : (c + 1) * 512, : T * 128]  # [512 f, T*128 l]
        subT = sub.T.reshape(T, 128, 512)  # [T, 128 l, 512 f]
        blocks.append(subT.transpose(1, 0, 2).reshape(128, T * 512))
    return np.ascontiguousarray(np.concatenate(blocks, axis=1)).astype(ml_dtypes.bfloat16)


def kernel(x, Wp, bp, Wc, bc, Wo, bo):
    global LAST_EXEC_NS
    x = np.asarray(x, dtype=np.float32)
    Wp = np.asarray(Wp, dtype=np.float32)
    bp = np.asarray(bp, dtype=np.float32)
    Wc = np.asarray(Wc, dtype=np.float32)
    bc = np.asarray(bc, dtype=np.float32)
    Wo = np.asarray(Wo, dtype=np.float32)
    bo = np.asarray(bo, dtype=np.float32)

    nc = _build_program()

    WoT = np.ascontiguousarray(Wo.T)  # [din, dout]
    wc_packed = [_pack_wc_head(Wc[h]) for h in range(H)]
    wpT_pair = []
    woT_pair = []
    for hp in range(2):
        h0, h1 = 2 * hp, 2 * hp + 1
        wpT = np.concatenate([Wp[h0].T, Wp[h1].T], axis=1)  # [D, 2E]
        # pre-tile: wpTt[p, d*512 + e] = wpT[d*128+p, e]
        wpT_pair.append(
            np.ascontiguousarray(
                wpT.reshape(DT8, 128, 2 * E).transpose(1, 0, 2).reshape(128, -1)
            ).astype(ml_dtypes.bfloat16)
        )
        woT_h = np.concatenate(
            [WoT[h0 * E : (h0 + 1) * E], WoT[h1 * E : (h1 + 1) * E]], axis=0
        )  # [2E, D]
        # pre-tile: woTt[p, t*D + j] = woT[t*128+p, j]
        woT_pair.append(
            np.ascontiguousarray(
                woT_h.reshape(4, 128, D).transpose(1, 0, 2).reshape(128, -1)
            ).astype(ml_dtypes.bfloat16)
        )

    # pre-tile x: xTt[p, c*4096 + t*512 + l] = x[b][c*512+l, t*128+p]
    xTt_b = [
        np.ascontiguousarray(
            x[b].T.reshape(DT8, 128, FC, 512).transpose(1, 2, 0, 3).reshape(128, -1)
        ).astype(ml_dtypes.bfloat16)
        for b in range(B)
    ]

    in_maps = []
    for c in range(8):
        b, hp = c // 2, c % 2
        in_maps.append(
            {
                "xTt": xTt_b[b],
                "wpTt": wpT_pair[hp],
                "wc0": wc_packed[2 * hp],
                "wc1": wc_packed[2 * hp + 1],
                "woTt": woT_pair[hp],
            }
        )

    res = run_bass_kernel_spmd(
        nc, in_maps, core_ids=list(range(8)), trace=TRACE
    )
    LAST_EXEC_NS = res.exec_time_ns

    # Host: fold all bias terms into one [L, D] matrix.
    # mixed bias = tril-rowsum(Wc)[h,f] * bp[h,e] + bc[h,f]; through Wo:
    rs = np.tril(Wc).sum(axis=2)  # [H, L]
    Wo_hE = Wo.reshape(D, H, E)
    V = np.einsum("he,jhe->hj", bp, Wo_hE)  # [H, D]
    WoSum = Wo_hE.sum(axis=2)  # [D, H]
    bias_total = rs.T @ V + bc.T @ WoSum.T + bo[None, :]  # [L, D]

    out = np.empty((B, L, D), dtype=np.float32)
    for b in range(B):
        out[b] = (
            res.results[2 * b]["part"].astype(np.float32)
            + res.results[2 * b + 1]["part"].astype(np.float32)
            + bias_total
        )
    return out


# revision 24
# speedup vs baseline: 1.3084x; 1.0113x over previous
"""MixerHead kernel for 8 trn2 NeuronCores (Bass/Tile, bf16 matmuls).

Math (reference):
  proj[b,h,l,e]  = sum_d x[b,l,d] Wp[h,e,d] + bp[h,e]
  mixed[b,h,f,e] = sum_{l<=f} Wc[h,f,l] proj[b,h,l,e] + bc[h,f]
  out[b,f,j]     = sum_{h,e} mixed[b,h,f,e] Wo[j, h*E+e] + bo[j]

Sharding: core c = (batch b = c//2, head-pair hp = c%2 -> heads {2hp, 2hp+1}).
Each core computes the bias-free linear part for its (batch, 2 heads) and
writes a partial [L, D] output; host sums the two partials per batch and adds
all bias contributions (folded into a single [L, D] matrix analytically).

Device layout chain (every matmul is out = lhsT.T @ rhs, contraction on the
partition dim):
  phase1: proj[l,e]    lhsT = xT[d, l-tile]          rhs = WpT[d, e(512)]
  phase2: mixedT[e,f]  lhsT = proj[l-tile, e-block]  rhs = WcT[l-tile, f-chunk]
          (WcT is pre-masked tril(Wc).T, packed on host so only lower-tri
           l-tiles are stored/loaded/computed; within a chunk the last 3
           l-tiles use a shrinking free dim to skip the masked-out f-tiles)
  phase3: part[f,dout] lhsT = mixT[e-blk, f-tile]    rhs = WoT[e-blk, dout]

Schedule: phase1 for all 4 l-chunks first, then f-chunks in DESCENDING size
order (3,2,1,0) so the final phase2->phase3->store tail is the smallest
chunk.  All weights (wc both heads, wo) are SBUF-resident, prefetched with
big DMAs issued at kernel start in exactly consumption order.
"""

import sys

for _p in ("/opt/trn_rl_repo", "/root/.axon_site/_ro/trn_rl_repo"):
    if _p not in sys.path:
        sys.path.append(_p)

import numpy as np

import ml_dtypes

try:  # make trace requests degrade gracefully if the NTFF hook module is absent
    import antenv.axon_hooks  # noqa: F401
except ImportError:
    import types

    import antenv

    _m = types.ModuleType("antenv.axon_hooks")
    _h = {}
    _m.set_axon_ntff_profile_hook = lambda hook: _h.__setitem__("h", hook)
    _m.get_axon_ntff_profile_hook = lambda: _h.get("h")
    sys.modules["antenv.axon_hooks"] = _m
    antenv.axon_hooks = _m

from concourse import bacc, mybir, tile
from concourse.bass_utils import run_bass_kernel_spmd

B, L, D, H, E = 4, 2048, 1024, 4, 256
F32 = mybir.dt.float32
BF16 = mybir.dt.bfloat16

LT = L // 128   # 16 l-tiles per batch
FC = 4          # f-chunks of 512
DT8 = D // 128  # 8 d-tiles
CHUNK_ORDER = (3, 2, 1, 0)  # descending causal depth
WC_PACK_COLS = sum((4 * c + 4) * 512 for c in range(FC))  # 20480
# col offset of chunk c's block in the descending-packed wc
WC_OFFS = {}
_off = 0
for _c in CHUNK_ORDER:
    WC_OFFS[_c] = _off
    _off += (4 * _c + 4) * 512

# Set by test harness: run with trace and record exec time.
TRACE = False
LAST_EXEC_NS = None

_cache = {}


def _build_program():
    if "nc" in _cache:
        return _cache["nc"]
    nc = bacc.Bacc("TRN2", target_bir_lowering=False, debug=False, num_devices=8)

    # All inputs pre-tiled on host to the exact SBUF layout ([128, N]) so
    # every DMA is a contiguous column slice (2-8KB per-partition runs ->
    # large descriptors -> full HBM bandwidth during the startup burst).
    xTt = nc.dram_tensor("xTt", [128, 4 * DT8 * 512], BF16, kind="ExternalInput")
    wpTt = nc.dram_tensor("wpTt", [128, DT8 * 512], BF16, kind="ExternalInput")
    wc0 = nc.dram_tensor("wc0", [128, WC_PACK_COLS], BF16, kind="ExternalInput")
    wc1 = nc.dram_tensor("wc1", [128, WC_PACK_COLS], BF16, kind="ExternalInput")
    woTt = nc.dram_tensor("woTt", [128, 4 * D], BF16, kind="ExternalInput")
    part = nc.dram_tensor("part", [L, D], BF16, kind="ExternalOutput")
    wc_dram = [wc0, wc1]

    with tile.TileContext(nc) as tc:
        with (
            tc.tile_pool(name="wp", bufs=1) as wp_pool,
            tc.tile_pool(name="wo", bufs=1) as wo_pool,
            tc.tile_pool(name="xt", bufs=1) as x_pool,
            tc.tile_pool(name="wc", bufs=1) as wc_pool,
            tc.tile_pool(name="proj", bufs=1) as proj_pool,
            tc.tile_pool(name="mix", bufs=1) as mix_pool,
            tc.tile_pool(name="outs", bufs=4) as out_pool,
            tc.tile_pool(name="ps1", bufs=1, space="PSUM") as ps1_pool,
            tc.tile_pool(name="ps2", bufs=2, space="PSUM") as ps2_pool,
            tc.tile_pool(name="ps3", bufs=2, space="PSUM") as ps3_pool,
        ):
            # PE warm-up: dummy matmuls with no DMA dependency run during the
            # startup loads so the HAM clock-gate opens (1.2 -> 2.4 GHz)
            # before the first real matmul.
            warm = wp_pool.tile([128, 512], BF16, tag="warm")
            # memset on vector: its preamble ends ~0.7us before gpsimd's,
            # so the warmup matmuls (and the whole PE stream) start earlier
            nc.vector.memset(warm[:], 0.0)
            # Warm-ups bridge from preamble end (~7.6us) past the first
            # DMA-piece arrival (~12us); HAM flips to 2.4GHz ~3.4us in, so
            # the real matmul stream starts fully warm with no early gaps
            # (any gap before the flip restarts the 3.4us busy window).
            ps_w = ps3_pool.tile([128, 512], F32, tag="ps3", name="ps_warm")
            for _ in range(12):
                nc.tensor.matmul(
                    ps_w[:], warm[:, :128], warm[:], start=True, stop=True
                )

            # ---- resident weights / inputs, prefetched in consumption order.
            # Two HWDGE queues (sync, scalar); pieces assigned so each queue
            # delivers in PE-need order.
            wp_all = wp_pool.tile([128, DT8 * 2 * E], BF16, tag="wp")
            for s, w in ((0, 1024), (1024, 1024), (2048, 2048)):  # d0-1, d2-3, d4-7
                nc.sync.dma_start(wp_all[:, s : s + w], wpTt[:, s : s + w])
            wp = [wp_all[:, d * 512 : (d + 1) * 512] for d in range(DT8)]

            xt_tiles = {}

            def load_xt(c, pieces, eng):
                xt_all = x_pool.tile(
                    [128, DT8 * 512], BF16, tag=f"xt{c}", name=f"xt_{c}"
                )
                xt_tiles[c] = xt_all
                for s, w in pieces:
                    eng.dma_start(
                        xt_all[:, s : s + w],
                        xTt[:, c * 4096 + s : c * 4096 + s + w],
                    )

            load_xt(0, ((0, 1024), (1024, 1024), (2048, 2048)), nc.scalar)
            load_xt(1, ((0, 2048), (2048, 2048)), nc.sync)
            load_xt(2, ((0, 2048), (2048, 2048)), nc.scalar)
            load_xt(3, ((0, 2048), (2048, 2048)), nc.scalar)

            wc_sb = [
                wc_pool.tile([128, WC_PACK_COLS], BF16, tag=f"wch{hh}", name=f"wc_{hh}")
                for hh in range(2)
            ]
            # (col_start, col_width) pieces per head: chunk 3 split in two,
            # then chunks 2, 1, 0.
            wc_pieces = [
                (WC_OFFS[3], 4096),
                (WC_OFFS[3] + 4096, 4096),
                (WC_OFFS[2], 6144),
                (WC_OFFS[1], 4096),
                (WC_OFFS[0], 2048),
            ]
            for s, w in wc_pieces:  # head 0 on sync (behind wp + all xt)
                nc.sync.dma_start(wc_sb[0][:, s : s + w], wc_dram[0][:, s : s + w])

            # head 1 chunk-3 pieces, then wo, then the rest (scalar queue)
            for s, w in wc_pieces[:2]:
                nc.scalar.dma_start(wc_sb[1][:, s : s + w], wc_dram[1][:, s : s + w])
            wo_all = wo_pool.tile([128, 4 * D], BF16, tag="wo", name="wo_all")
            nc.scalar.dma_start(wo_all[:], woTt[:, :])
            for s, w in wc_pieces[2:]:
                nc.scalar.dma_start(wc_sb[1][:, s : s + w], wc_dram[1][:, s : s + w])

            proj = [None] * LT
            mix = [[None] * FC for _ in range(4)]

            def phase1(c):
                ps1 = [
                    ps1_pool.tile([128, 2 * E], F32, tag=f"ps1_{i}", name=f"ps1_{c}_{i}")
                    for i in range(4)
                ]
                xt_all = xt_tiles[c]
                for d in range(DT8):
                    for i in range(4):
                        nc.tensor.matmul(
                            ps1[i][:],
                            xt_all[:, d * 512 + i * 128 : d * 512 + (i + 1) * 128],
                            wp[d],
                            start=(d == 0),
                            stop=(d == DT8 - 1),
                        )
                for i in range(4):
                    lt = c * 4 + i
                    pt = proj_pool.tile(
                        [128, 2 * E], BF16, tag=f"proj{lt}", name=f"proj_{lt}"
                    )
                    nc.vector.tensor_copy(pt[:], ps1[i][:])
                    proj[lt] = pt

            def phase2(c):
                # causal => l-tiles 0..4c+3 (mask pre-applied in the packed Wc)
                # l-tile t covers f-tiles >= t-4c within the chunk, so the
                # last three l-tiles shrink their free dim.
                T = 4 * c + 4
                base = WC_OFFS[c]
                for hh in range(2):
                    wct = wc_sb[hh]
                    for eb in (2 * hh, 2 * hh + 1):
                        ps = ps2_pool.tile(
                            [128, 512], F32, tag="ps2", name=f"ps2_{c}_{eb}"
                        )
                        for t in range(T):
                            sh = 128 * max(0, t - 4 * c)
                            nc.tensor.matmul(
                                ps[:, sh:],
                                proj[t][:, eb * 128 : (eb + 1) * 128],
                                wct[:, base + t * 512 + sh : base + (t + 1) * 512],
                                start=(t == 0),
                                stop=(t == T - 1),
                            )
                        mt = mix_pool.tile(
                            [128, 512], BF16, tag=f"m{eb}_{c}", name=f"mix_{eb}_{c}"
                        )
                        # the last chain gates phase3: copy it on the (fast)
                        # vector engine, earlier ones on scalar
                        if eb == 3:
                            nc.vector.tensor_copy(mt[:], ps[:])
                        else:
                            nc.scalar.copy(mt[:], ps[:])
                        mix[eb][c] = mt

            def phase3(c):
                for fi in range(4):
                    ft = c * 4 + fi
                    ot = out_pool.tile([128, D], BF16, tag="out", name=f"out_{ft}")
                    for dc in range(2):
                        ps = ps3_pool.tile(
                            [128, 512], F32, tag="ps3", name=f"ps3_{ft}_{dc}"
                        )
                        for eb in range(4):
                            nc.tensor.matmul(
                                ps[:],
                                mix[eb][c][:, fi * 128 : (fi + 1) * 128],
                                wo_all[
                                    :, eb * D + dc * 512 : eb * D + (dc + 1) * 512
                                ],
                                start=(eb == 0),
                                stop=(eb == 3),
                            )
                        nc.vector.tensor_copy(ot[:, dc * 512 : (dc + 1) * 512], ps[:])
                    nc.scalar.dma_start(part[ft * 128 : (ft + 1) * 128, :], ot[:])

            # Chunk 0 is processed in two 256-f halves so that only ~half a
            # phase3 group remains after the very last phase2 chain -- this
            # shortens the kernel tail (final out-DMA receipt gates the
            # closing barrier).
            mix0 = [[None, None] for _ in range(4)]  # [eb][half]

            def phase2_half0(h):
                base = WC_OFFS[0]
                for hh in range(2):
                    wct = wc_sb[hh]
                    for eb in (2 * hh, 2 * hh + 1):
                        ps = ps2_pool.tile(
                            [128, 256], F32, tag="ps2", name=f"ps2h_{h}_{eb}"
                        )
                        for t in range(2 * h + 2):
                            sh = 128 * max(0, t - 2 * h)
                            nc.tensor.matmul(
                                ps[:, sh:],
                                proj[t][:, eb * 128 : (eb + 1) * 128],
                                wct[
                                    :,
                                    base + t * 512 + 256 * h + sh : base
                                    + t * 512
                                    + 256 * (h + 1),
                                ],
                                start=(t == 0),
                                stop=(t == 2 * h + 1),
                            )
                        mt = mix_pool.tile(
                            [128, 256], BF16, tag=f"m0h{eb}_{h}", name=f"mix0_{eb}_{h}"
                        )
                        if eb == 3:
                            nc.vector.tensor_copy(mt[:], ps[:])
                        else:
                            nc.scalar.copy(mt[:], ps[:])
                        mix0[eb][h] = mt

            def phase3_half0(h):
                for fi in (2 * h, 2 * h + 1):
                    last = h == 1 and fi == 3
                    ot = out_pool.tile([128, D], BF16, tag="out", name=f"out_{fi}")
                    for dc in range(2):
                        if last:
                            # final output: two n=256 chains + split copies and
                            # stores so the very last store issues ASAP after
                            # the last matmul (its HBM receipt gates the
                            # closing barrier)
                            for q in range(2):
                                ps = ps3_pool.tile(
                                    [128, 256], F32, tag="ps3", name=f"ps3f_{dc}_{q}"
                                )
                                for eb in range(4):
                                    nc.tensor.matmul(
                                        ps[:],
                                        mix0[eb][h][:, 128 * (fi - 2 * h) : 128 * (fi - 2 * h + 1)],
                                        wo_all[
                                            :,
                                            eb * D + dc * 512 + q * 256 : eb * D
                                            + dc * 512
                                            + (q + 1) * 256,
                                        ],
                                        start=(eb == 0),
                                        stop=(eb == 3),
                                    )
                                lo = dc * 512 + q * 256
                                nc.vector.tensor_copy(ot[:, lo : lo + 256], ps[:])
                                # alternate issue engines so the last store's
                                # ~0.6us issue slot doesn't queue behind the
                                # previous piece's
                                eng = nc.sync if q == 0 else nc.scalar
                                eng.dma_start(
                                    part[fi * 128 : (fi + 1) * 128, lo : lo + 256],
                                    ot[:, lo : lo + 256],
                                )
                        else:
                            ps = ps3_pool.tile(
                                [128, 512], F32, tag="ps3", name=f"ps3_{fi}_{dc}"
                            )
                            for eb in range(4):
                                nc.tensor.matmul(
                                    ps[:],
                                    mix0[eb][h][:, (fi - 2 * h) * 128 : (fi - 2 * h + 1) * 128],
                                    wo_all[
                                        :, eb * D + dc * 512 : eb * D + (dc + 1) * 512
                                    ],
                                    start=(eb == 0),
                                    stop=(eb == 3),
                                )
                            nc.vector.tensor_copy(ot[:, dc * 512 : (dc + 1) * 512], ps[:])
                    if not last:
                        nc.scalar.dma_start(part[fi * 128 : (fi + 1) * 128, :], ot[:])

            for c in range(FC):
                phase1(c)
            for c in CHUNK_ORDER[:3]:
                phase2(c)
                phase3(c)
            phase2_half0(0)
            phase3_half0(0)
            phase2_half0(1)
            phase3_half0(1)

    nc.compile()
    _cache["nc"] = nc
    return nc


def _pack_wc_head(wc_h: np.ndarray) -> np.ndarray:
    """tril(Wc[h]) -> [128, 20480]: per f-chunk c (descending order 3,2,1,0),
    the l-tiles 0..4c+3 of WcT = tril(Wc).T laid out as
    [128 l-partitions, T*512 f-cols]."""
    m = np.tril(wc_h)  # [f, l]
    blocks = []
    for c in CHUNK_ORDER:
        T = 4 * c + 4
        sub = m[c * 512 : (c + 1) * 512, : T * 128]  # [512 f, T*128 l]
        subT = sub.T.reshape(T, 128, 512)  # [T, 128 l, 512 f]
        blocks.append(subT.transpose(1, 0, 2).reshape(128, T * 512))
    return np.ascontiguousarray(np.concatenate(blocks, axis=1)).astype(ml_dtypes.bfloat16)


def kernel(x, Wp, bp, Wc, bc, Wo, bo):
    global LAST_EXEC_NS
    x = np.asarray(x, dtype=np.float32)
    Wp = np.asarray(Wp, dtype=np.float32)
    bp = np.asarray(bp, dtype=np.float32)
    Wc = np.asarray(Wc, dtype=np.float32)
    bc = np.asarray(bc, dtype=np.float32)
    Wo = np.asarray(Wo, dtype=np.float32)
    bo = np.asarray(bo, dtype=np.float32)

    nc = _build_program()

    WoT = np.ascontiguousarray(Wo.T)  # [din, dout]
    wc_packed = [_pack_wc_head(Wc[h]) for h in range(H)]
    wpT_pair = []
    woT_pair = []
    for hp in range(2):
        h0, h1 = 2 * hp, 2 * hp + 1
        wpT = np.concatenate([Wp[h0].T, Wp[h1].T], axis=1)  # [D, 2E]
        # pre-tile: wpTt[p, d*512 + e] = wpT[d*128+p, e]
        wpT_pair.append(
            np.ascontiguousarray(
                wpT.reshape(DT8, 128, 2 * E).transpose(1, 0, 2).reshape(128, -1)
            ).astype(ml_dtypes.bfloat16)
        )
        woT_h = np.concatenate(
            [WoT[h0 * E : (h0 + 1) * E], WoT[h1 * E : (h1 + 1) * E]], axis=0
        )  # [2E, D]
        # pre-tile: woTt[p, t*D + j] = woT[t*128+p, j]
        woT_pair.append(
            np.ascontiguousarray(
                woT_h.reshape(4, 128, D).transpose(1, 0, 2).reshape(128, -1)
            ).astype(ml_dtypes.bfloat16)
        )

    # pre-tile x: xTt[p, c*4096 + t*512 + l] = x[b][c*512+l, t*128+p]
    xTt_b = [
        np.ascontiguousarray(
            x[b].T.reshape(DT8, 128, FC, 512).transpose(1, 2, 0, 3).reshape(128, -1)
        ).astype(ml_dtypes.bfloat16)
        for b in range(B)
    ]

    in_maps = []
    for c in range(8):
        b, hp = c // 2, c % 2
        in_maps.append(
            {
                "xTt": xTt_b[b],
                "wpTt": wpT_pair[hp],
                "wc0": wc_packed[2 * hp],
                "wc1": wc_packed[2 * hp + 1],
                "woTt": woT_pair[hp],
            }
        )

    res = run_bass_kernel_spmd(
        nc, in_maps, core_ids=list(range(8)), trace=TRACE
    )
    LAST_EXEC_NS = res.exec_time_ns

    # Host: fold all bias terms into one [L, D] matrix.
    # mixed bias = tril-rowsum(Wc)[h,f] * bp[h,e] + bc[h,f]; through Wo:
    rs = np.tril(Wc).sum(axis=2)  # [H, L]
    Wo_hE = Wo.reshape(D, H, E)
    V = np.einsum("he,jhe->hj", bp, Wo_hE)  # [H, D]
    WoSum = Wo_hE.sum(axis=2)  # [D, H]
    bias_total = rs.T @ V + bc.T @ WoSum.T + bo[None, :]  # [L, D]

    out = np.empty((B, L, D), dtype=np.float32)
    for b in range(B):
        out[b] = (
            res.results[2 * b]["part"].astype(np.float32)
            + res.results[2 * b + 1]["part"].astype(np.float32)
            + bias_total
        )
    return out
